# revision 6
# baseline (speedup 1.0000x reference)
"""Trainium2 Bass kernel for nn_Attention_30666066493686.

Math (per timestep t):
  fproj  = frame_t @ Wf + b_att                                   [B, A]
  hidden = tanh(region_t @ Wr + fproj)                            [N*B, A]
  att    = hidden . W_full   (+ b_full dropped: softmax-shift inv.)
  alpha  = softmax_n(where(mask, -1e9, att))
  out_t  = sum_n alpha * region_t                                 [B, D]

Sharding: data-parallel over T across 8 NeuronCores (4 timesteps each),
params replicated, no collectives.

Design notes:
  - The PE contracts over the partition dim, and region participates in two
    contractions over different axes (over d in phase 1, over n in phase 2),
    so the host sends region in BOTH orientations as bf16 - same total
    bytes as one f32 read, but no on-chip transposes at all:
      regN[t, p, c, d] = region[t, row=128c+p, d]   (rows on partitions)
      regT[t, p, j, r] = region[t, row=r, d=4p+j]   (d on partitions)
    Both are partition-major so each DMA line is one contiguous ~18KB
    descriptor (near-peak HBM bandwidth).
  - rows = (n,b) flattened = 2304 = 18 chunks of 128; row = 128c+p makes
    b = row % 64 = p % 64, invariant in c - so the softmax fold and the
    phase-2 diagonal patterns are c-independent constants.
  - Phase 1: z^T[A, rows] = Wr^T @ region^T with lhsT = permuted-Wr chunks,
    rhs = regT slices; the (fproj + b_att) bias is folded in as one K=64
    matmul per 512-column group against a tiled eye(64) pattern.
  - att columnized on PE (lhsT = tanh chunk, rhs = W_full) so softmax runs
    partition-parallel; exp without max-subtraction (|att| <= ~12); mask
    applied as a 0/1 multiply after exp; normalization folded into the
    output scale.
  - Phase 2 on PE: po[64, 512] = sum_c dg_c^T @ regN_c with
    dg_c[p, b] = expm[p, c] * [p%64 == b].
  - Loads stream on the SP HWDGE ring; the out-store issues on the ACT
    HWDGE ring so it cannot block the next timestep's loads (HWDGE rings
    are strict FIFO per issuing engine).
  - The timing loop (iters > 1) unrolls 12 bodies per For_i trip to
    amortize the ~2us all-engine back-edge barrier and pipeline fill.
"""

import ml_dtypes
import numpy as np

T, N, B, D, A = 32, 36, 64, 512, 128
N_CORES = 8
T_LOC = T // N_CORES           # 4
ROWS = N * B                   # 2304
NCH = ROWS // 128              # 18
GROUPS = [(0, 512), (512, 512), (1024, 512), (1536, 512), (2048, 256)]

_NC_CACHE = {}


def _build_nc(iters=1, unroll=12, bufs_big=3, phh_bufs=4, diag_bufs=4):
    import concourse.bacc as bacc
    import concourse.bass as bass
    from concourse import mybir
    from concourse.tile import TileContext

    f32 = mybir.dt.float32
    bf16 = mybir.dt.bfloat16
    AF = mybir.ActivationFunctionType

    nc = bacc.Bacc(
        "TRN2", target_bir_lowering=False, debug=False, num_devices=N_CORES
    )
    regN = nc.dram_tensor("regN", [T_LOC, 128, NCH, D], bf16, kind="ExternalInput")
    f8 = mybir.dt.float8e4
    regT = nc.dram_tensor("regT", [T_LOC, 128, 4, ROWS], f8, kind="ExternalInput")
    frameT = nc.dram_tensor("frameT", [128, T_LOC, 4, B], bf16, kind="ExternalInput")
    maskh = nc.dram_tensor(
        "maskh", [128, T_LOC, NCH], mybir.dt.uint8, kind="ExternalInput"
    )
    wrow = nc.dram_tensor("wrow", [128, 4, A], mybir.dt.float8e4, kind="ExternalInput")
    wf = nc.dram_tensor("wf", [128, 4, A], bf16, kind="ExternalInput")
    wfull = nc.dram_tensor("wfull", [A, 1], bf16, kind="ExternalInput")
    batt = nc.dram_tensor("batt", [1, A], f32, kind="ExternalInput")
    i64 = nc.dram_tensor("i64", [64, 64], bf16, kind="ExternalInput")
    diag01 = nc.dram_tensor("diag01", [128, 64], f32, kind="ExternalInput")
    diag01b = nc.dram_tensor("diag01b", [128, 64], bf16, kind="ExternalInput")
    ones_row = nc.dram_tensor("ones_row", [1, 64], f32, kind="ExternalInput")
    out = nc.dram_tensor("out", [T_LOC, B, D], f32, kind="ExternalOutput")

    with TileContext(nc) as tc:
        with (
            tc.tile_pool(name="consts", bufs=1) as consts,
            tc.tile_pool(name="rnp", bufs=bufs_big) as rnp,
            tc.tile_pool(name="rtp", bufs=bufs_big) as rtp,
            tc.tile_pool(name="thp", bufs=2) as thp,
            tc.tile_pool(name="smallp", bufs=3) as smallp,
            tc.tile_pool(name="diagp", bufs=diag_bufs) as diagp,
            tc.tile_pool(name="outp", bufs=2) as outp,
            tc.tile_pool(name="phh", bufs=phh_bufs, space="PSUM") as phh,
            tc.tile_pool(name="psmall", bufs=2, space="PSUM") as psmall,
            tc.tile_pool(name="po", bufs=2, space="PSUM") as po,
        ):
            # ---- constants (loaded once) ----
            wrow_sb = consts.tile([128, 4, A], mybir.dt.float8e4)
            nc.sync.dma_start(out=wrow_sb, in_=wrow.ap())
            wf_sb = consts.tile([128, 4, A], bf16)
            nc.sync.dma_start(out=wf_sb, in_=wf.ap())
            wfull_sb = consts.tile([128, 1], bf16)
            nc.sync.dma_start(out=wfull_sb, in_=wfull.ap())
            batt_sb = consts.tile([1, A], f32)
            nc.sync.dma_start(out=batt_sb, in_=batt.ap())
            i64_sb = consts.tile([64, 64], bf16)
            nc.sync.dma_start(out=i64_sb, in_=i64.ap())
            diag01_sb = consts.tile([128, 64], f32)
            nc.sync.dma_start(out=diag01_sb, in_=diag01.ap())
            diag01b_sb = consts.tile([128, 64], bf16)
            nc.sync.dma_start(out=diag01b_sb, in_=diag01b.ap())
            onesr_sb = consts.tile([1, 64], f32)
            nc.sync.dma_start(out=onesr_sb, in_=ones_row.ap())

            def body():
                ft_all = smallp.tile([128, T_LOC, 4, B], bf16, tag="ft", name="ftA")
                nc.sync.dma_start(out=ft_all, in_=frameT.ap())
                mk_all = smallp.tile(
                    [128, T_LOC, NCH], mybir.dt.uint8, tag="mk", name="mkA"
                )
                nc.sync.dma_start(out=mk_all, in_=maskh.ap())
                for t in range(T_LOC):
                    # ---- loads for this t (SP ring: loads only) ----
                    rn = rnp.tile([128, NCH, D], bf16, tag="rn", name=f"rn{t}")
                    rt = rtp.tile([128, 4, ROWS], f8, tag="rt", name=f"rt{t}")
                    nc.sync.dma_start(out=rt, in_=regT.ap()[t])
                    nc.sync.dma_start(out=rn, in_=regN.ap()[t])
                    ft = ft_all[:, t]
                    mk = mk_all[:, t]
                    # keep = 1 - mask  (f32)
                    keep = smallp.tile([128, NCH], f32, tag="keep", name=f"kp{t}")
                    nc.scalar.activation(
                        out=keep, in_=mk, func=AF.Identity, bias=1.0, scale=-1.0
                    )

                    # ---- fproj[b, a] = frame_t @ Wf + b_att ----
                    pf = psmall.tile([64, A], f32, tag="s", name=f"pf{t}")
                    for j in range(4):
                        nc.tensor.matmul(
                            pf,
                            lhsT=ft[:, j, :],
                            rhs=wf_sb[:, j, :],
                            start=(j == 0),
                            stop=False,
                        )
                    nc.tensor.matmul(
                        pf, lhsT=onesr_sb, rhs=batt_sb, start=False, stop=True
                    )
                    fpb = smallp.tile([64, A], bf16, tag="fpb", name=f"fp{t}")
                    nc.scalar.copy(out=fpb, in_=pf)

                    # ---- phase 1: z^T = Wr^T @ region^T + bias; tanh ----
                    th = thp.tile([128, ROWS], bf16, tag="th", name=f"th{t}")
                    for g, (c0, cw) in enumerate(GROUPS):
                        ph = phh.tile([128, 512], f32, tag="ph", name=f"ph{t}_{g}")
                        for jj in range(2):
                            nc.tensor.matmul(
                                ph[:, :cw],
                                lhsT=wrow_sb[:, 2 * jj : 2 * jj + 2, :],
                                rhs=rt[:, 2 * jj : 2 * jj + 2, c0 : c0 + cw],
                                start=(jj == 0),
                                stop=False,
                                perf_mode=mybir.MatmulPerfMode.DoubleRow,
                            )
                        reps = cw // 64
                        i64t = bass.AP(
                            tensor=i64_sb.tensor,
                            offset=i64_sb.offset,
                            ap=[list(i64_sb.ap[0]), [0, reps], list(i64_sb.ap[1])],
                        )
                        nc.tensor.matmul(
                            ph[:, :cw], lhsT=fpb, rhs=i64t, start=False, stop=True
                        )
                        nc.scalar.activation(
                            out=th[:, c0 : c0 + cw], in_=ph[:, :cw], func=AF.Tanh
                        )

                    # ---- att columnized: patt[p, c] = att[row 128c+p] ----
                    patt = psmall.tile([128, NCH], f32, tag="s", name=f"pa{t}")
                    for c in range(NCH):
                        nc.tensor.matmul(
                            patt[:, c : c + 1],
                            lhsT=th[:, c * 128 : (c + 1) * 128],
                            rhs=wfull_sb,
                            start=True,
                            stop=True,
                        )

                    # ---- masked softmax pieces ----
                    expr = smallp.tile([128, NCH], f32, tag="expr", name=f"ex{t}")
                    nc.scalar.activation(out=expr, in_=patt, func=AF.Exp)
                    expm = smallp.tile([128, NCH], f32, tag="expm", name=f"em{t}")
                    nc.vector.tensor_mul(expm, expr, keep)
                    sacc = smallp.tile([128, 1], f32, tag="sacc", name=f"sa{t}")
                    nc.vector.tensor_reduce(
                        out=sacc,
                        in_=expm,
                        axis=mybir.AxisListType.X,
                        op=mybir.AluOpType.add,
                    )
                    ps64 = psmall.tile([64, 1], f32, tag="s", name=f"ps{t}")
                    nc.tensor.matmul(
                        ps64, lhsT=diag01_sb, rhs=sacc, start=True, stop=True
                    )
                    rs = smallp.tile([64, 1], f32, tag="rs", name=f"rs{t}")
                    nc.vector.reciprocal(out=rs, in_=ps64)

                    # ---- phase 2: po = sum_c dg_c^T @ regN_c ----
                    po_t = po.tile([64, 512], f32, tag="po", name=f"po{t}")
                    for c in range(NCH):
                        dg = diagp.tile([128, 64], bf16, tag="dg", name=f"dg{t}_{c}")
                        nc.vector.tensor_scalar_mul(
                            out=dg, in0=diag01b_sb, scalar1=expm[:, c : c + 1]
                        )
                        nc.tensor.matmul(
                            po_t,
                            lhsT=dg,
                            rhs=rn[:, c, :],
                            start=(c == 0),
                            stop=(c == NCH - 1),
                        )
                    osb = outp.tile([64, 512], f32, tag="osb", name=f"ob{t}")
                    nc.vector.tensor_scalar_mul(out=osb, in0=po_t, scalar1=rs)
                    # store on the ACT ring so it never blocks SP-ring loads
                    nc.scalar.dma_start(out=out.ap()[t], in_=osb)

            if iters == 1:
                body()
            else:
                trips, rem = divmod(iters, unroll)
                with tc.For_i(0, trips, 1):
                    for _u in range(unroll):
                        body()
                for _u in range(rem):
                    body()

    nc.compile()
    return nc


def _get_nc(iters=1, unroll=12, bufs_big=3, phh_bufs=4, diag_bufs=4):
    key = (iters, unroll, bufs_big, phh_bufs, diag_bufs)
    if key not in _NC_CACHE:
        _NC_CACHE[key] = _build_nc(iters, unroll, bufs_big, phh_bufs, diag_bufs)
    return _NC_CACHE[key]


def _make_in_maps(region_feat, frame_feat, mask, W_att, b_att, W_full):
    bf = ml_dtypes.bfloat16
    region_feat = np.asarray(region_feat, np.float32)
    frame_feat = np.asarray(frame_feat, np.float32)
    mask = np.asarray(mask)

    Wr = np.asarray(W_att[:D], np.float32)   # [D, A]
    Wf = np.asarray(W_att[D:], np.float32)   # [D, A]
    # Wr permuted to regT's p-major d order: wrow[p, j, a] = Wr[4p+j, a]
    wrow = np.ascontiguousarray(Wr.reshape(128, 4, A)).astype(
        ml_dtypes.float8_e4m3
    )
    # Wf in natural d-chunks matching frameT: wf[p, j, a] = Wf[128j+p, a]
    wfc = np.ascontiguousarray(Wf.reshape(4, 128, A).transpose(1, 0, 2)).astype(bf)

    diag01 = np.zeros((128, 64), np.float32)
    diag01[np.arange(128), np.arange(128) % 64] = 1.0
    consts = {
        "wrow": wrow,
        "wf": wfc,
        "wfull": np.ascontiguousarray(
            np.asarray(W_full, np.float32).reshape(A, 1)
        ).astype(bf),
        "batt": np.ascontiguousarray(
            np.asarray(b_att, np.float32).reshape(1, A)
        ),
        "i64": np.eye(64, dtype=np.float32).astype(bf),
        "diag01": diag01,
        "diag01b": diag01.astype(bf),
        "ones_row": np.ones((1, 64), np.float32),
    }

    in_maps = []
    for core in range(N_CORES):
        sl = slice(core * T_LOC, (core + 1) * T_LOC)
        reg = region_feat[sl].reshape(T_LOC, ROWS, D)     # row = n*64+b
        regb = reg.astype(bf)
        # natural, partition-major: regN[t, p, c, d] = region[t, 128c+p, d]
        regN = np.ascontiguousarray(
            regb.reshape(T_LOC, NCH, 128, D).transpose(0, 2, 1, 3)
        )
        # transposed: regT[t, p, j, r] = region[t, r, 4p+j]  (fp8 e4m3,
        # quantized from f32 directly to avoid double rounding)
        regT = np.ascontiguousarray(
            reg.transpose(0, 2, 1).reshape(T_LOC, 128, 4, ROWS)
        ).astype(ml_dtypes.float8_e4m3)
        # frameT[p, t, j, b] = frame[t, b, 128j+p]
        frT = np.ascontiguousarray(
            frame_feat[sl].astype(bf).transpose(0, 2, 1)
            .reshape(T_LOC, 4, 128, B).transpose(2, 0, 1, 3)
        )
        # maskh[p, t, c] = mask[t, row 128c+p]
        mkh = np.ascontiguousarray(
            mask[sl].reshape(T_LOC, ROWS).astype(np.uint8)
            .reshape(T_LOC, NCH, 128).transpose(2, 0, 1)
        )
        in_maps.append(
            {"regN": regN, "regT": regT, "frameT": frT, "maskh": mkh, **consts}
        )
    return in_maps


def kernel(region_feat, frame_feat, mask, W_att, b_att, W_full, b_full=None):
    """Full-input entry point. b_full accepted but unused: softmax is
    invariant to a constant shift of the logits."""
    from concourse.bass_utils import run_bass_kernel_spmd

    nc = _get_nc()
    in_maps = _make_in_maps(region_feat, frame_feat, mask, W_att, b_att, W_full)
    res = run_bass_kernel_spmd(nc, in_maps, core_ids=list(range(N_CORES)))
    return np.concatenate(
        [res.results[c]["out"] for c in range(N_CORES)], axis=0
    ).astype(np.float32)
